# revision 29
# baseline (speedup 1.0000x reference)
"""Trainium2 Bass kernel for nn_Attention_16071767621814.

MobileViT-style attention block: 3x (depthwise3x3 conv + BN + 1x1 pointwise)
for q/k/v, 8-head attention (Lq=1024, Lkv=256, d=64), head-mixing reshape,
1x1 output projection.

Sharding: pure data-parallel over batch (16 batches / 8 cores = 2 per core),
zero collectives.

Per-core design (all layouts [feature-on-partition, token-on-free]):
- x DMA'd contiguously to a staging tile, then pad-converted to bf16 on
  Scalar/GpSimd (avoids fragmented padded-interior DMA descriptors).
- BN folded host-side: scale into dw weights, bias via pw const row.
- depthwise convs as 9 shifted scalar_tensor_tensor taps in bf16 on Vector.
- q tokens ordered i = m*128 + j (lq = j*8 + m) so the head-mixing
  reshape's m-phases are contiguous 128-blocks.
- pw matmuls at M=128 (two heads per matmul).
- S^T = k^T q via PE, exp on ScalarE, denominator via ones-lhsT matmul
  broadcast, normalize (avp*rc) on GpSimd -> avbf bf16.
- avbf refolded into F[kk][par*64+d, h*128+r] via 4 SBUF->SBUF DMAs/pair;
  o_proj is then 16 matmuls of N=512,K=128 per batch from F.
"""

import numpy as np
import ml_dtypes
BF16NP = ml_dtypes.bfloat16

from concourse import bass, bacc, tile, mybir
from concourse.bass_utils import run_bass_kernel_spmd

F32 = mybir.dt.float32
BF16 = mybir.dt.bfloat16
AF = mybir.ActivationFunctionType
OP = mybir.AluOpType

NCORES = 8
B, C, S = 16, 192, 32
BPC = B // NCORES          # 2 batches per core
Lq = S * S                 # 1024
Sk = S // 2                # 16
Lkv = Sk * Sk              # 256
HEADS, HID, INNER = 8, 64, 512
EPS = 1e-5
PS = S + 2                 # padded spatial 34

_NC = None
LAST_RESULT = None


def _build():
    nc = bacc.Bacc("TRN2", target_bir_lowering=False, debug=False,
                   num_devices=NCORES)

    x_ext = nc.declare_dram_parameter("x", [BPC, C, Lq], F32, isOutput=False)
    pw_ext = {}
    dw_ext = {}
    for p in ("q", "k", "v"):
        pw_ext[p] = nc.declare_dram_parameter(p + "pwT", [C + 1, INNER], BF16,
                                              isOutput=False)
        dw_ext[p] = nc.declare_dram_parameter(p + "dw", [C, 9], F32,
                                              isOutput=False)
    owt4_ext = nc.declare_dram_parameter("owt4", [128, 4 * C], BF16,
                                         isOutput=False)
    qdA_ext = nc.declare_dram_parameter("qdiagA", [128, 9 * 128], BF16,
                                        isOutput=False)
    qdB_ext = nc.declare_dram_parameter("qdiagB", [64, 9 * 64], BF16,
                                        isOutput=False)
    ones_ext = nc.declare_dram_parameter("ones128x64", [128, HID], BF16,
                                         isOutput=False)
    onesq_ext = nc.declare_dram_parameter("onesq", [1, BPC, Lq], BF16,
                                          isOutput=False)
    oneskv_ext = nc.declare_dram_parameter("oneskv", [1, BPC, Lkv], BF16,
                                           isOutput=False)
    ob_ext = nc.declare_dram_parameter("ob", [C, 1], F32, isOutput=False)
    out_ext = nc.declare_dram_parameter("out", [BPC, C, Lq], F32, isOutput=True)

    TAPS = [(dy, dx) for dy in range(3) for dx in range(3)]

    from contextlib import ExitStack
    with tile.TileContext(nc) as tc, ExitStack() as ctx:
        const = ctx.enter_context(tc.tile_pool(name="const", bufs=1))
        xpool = ctx.enter_context(tc.tile_pool(name="xpool", bufs=1))
        wpool = ctx.enter_context(tc.tile_pool(name="wpool", bufs=2))
        psw = ctx.enter_context(tc.tile_pool(name="psw", bufs=4, space="PSUM"))

        # ---- x staging DMA (contiguous, fast); batch 0 + diag weights
        # first so the prologue diag-conv isn't starved ----
        xsA = xpool.tile([128, BPC, Lq], F32, name="xsA")
        xsB = xpool.tile([C - 128, BPC, Lq], F32, name="xsB")
        nc.sync.dma_start(out=xsA[:, 0, :], in_=x_ext[0, 0:128, :])
        nc.sync.dma_start(out=xsB[:, 0, :], in_=x_ext[0, 128:C, :])
        qdA = const.tile([128, 9, 128], BF16, name="qdA")
        nc.sync.dma_start(out=qdA[:],
                          in_=qdA_ext[:].rearrange("p (t c) -> p t c", t=9))
        qdB = const.tile([64, 9, 64], BF16, name="qdB")
        nc.sync.dma_start(out=qdB[:],
                          in_=qdB_ext[:].rearrange("p (t c) -> p t c", t=9))

        # padded bf16 input tiles (zero borders only; interior overwritten)
        xpA = xpool.tile([128, BPC, PS, PS], BF16, name="xpA")
        xpB = xpool.tile([C - 128, BPC, PS, PS], BF16, name="xpB")
        for xp_t in (xpA, xpB):
            nc.vector.memset(xp_t[:, :, 0:1, :], 0.0)
            nc.vector.memset(xp_t[:, :, S + 1:S + 2, :], 0.0)
            nc.vector.memset(xp_t[:, :, :, 0:1], 0.0)
            nc.vector.memset(xp_t[:, :, :, S + 1:S + 2], 0.0)

        # phase-split planes for the stride-2 k/v convs: P[c,bi,py,px,u,v] =
        # xpad[c, bi, 2u+py, 2v+px]  (17 used rows, 18-wide for even stride)
        PU, PV = 17, 18
        phA = xpool.tile([128, BPC, 2, 2, PU, PV], BF16, name="phA")
        phB = xpool.tile([C - 128, BPC, 2, 2, PU, PV], BF16, name="phB")
        for ph_t in (phA, phB):
            # only pad row u=0 (py=0 planes) and pad col v=0 (px=0 planes)
            # are ever read by taps; zero just those.
            nc.vector.memset(ph_t[:, :, 0, :, 0:1, :], 0.0)
            nc.vector.memset(ph_t[:, :, :, 0, :, 0:1], 0.0)

        def emit_convert(bi):
            nc.scalar.copy(
                xpA[:, bi, 1:S + 1, 1:S + 1],
                xsA[:, bi].rearrange("p (h w) -> p h w", h=S))
            nc.scalar.copy(
                xpB[:, bi, 1:S + 1, 1:S + 1],
                xsB[:, bi].rearrange("p (h w) -> p h w", h=S))

        def emit_planes(bi):
            for (xs_t, ph_t) in ((xsA, phA), (xsB, phB)):
                np_ = min(xs_t.shape[0], 128)
                src = xs_t[0:np_, bi].rearrange("p (h w) -> p h w", h=S)
                for py in range(2):
                    for px in range(2):
                        u0 = 1 - py
                        v0 = 1 - px
                        r0 = 1 if py == 0 else 0
                        c0 = 1 if px == 0 else 0
                        dst = ph_t[0:np_, bi, py, px, u0:u0 + 16, v0:v0 + 16]
                        srcv = src[:, r0:r0 + 31:2, c0:c0 + 31:2]
                        if bi == 0:
                            nc.scalar.copy(dst, srcv)
                        else:
                            nc.gpsimd.tensor_copy(dst, srcv)

        emit_convert(0)
        emit_planes(0)

        # ---- weights to SBUF ----
        pwA, pwB, dwA, dwB = {}, {}, {}, {}
        for p in ("q", "k", "v"):
            pwA[p] = const.tile([128, INNER], BF16, name=f"pwA{p}")
            pwB[p] = const.tile([C + 1 - 128, INNER], BF16, name=f"pwB{p}")
            nc.sync.dma_start(out=pwA[p][:], in_=pw_ext[p][0:128, :])
            nc.sync.dma_start(out=pwB[p][:], in_=pw_ext[p][128:C + 1, :])
            dwA[p] = const.tile([128, 9], F32, name=f"dwA{p}")
            dwB[p] = const.tile([C - 128, 9], F32, name=f"dwB{p}")
            nc.sync.dma_start(out=dwA[p][:], in_=dw_ext[p][0:128, :])
            nc.sync.dma_start(out=dwB[p][:], in_=dw_ext[p][128:C, :])
        owt4 = const.tile([128, 4, C], BF16, name="owt4")
        nc.sync.dma_start(out=owt4[:],
                          in_=owt4_ext[:].rearrange("p (k c) -> p k c", k=4))
        # batch-1 staging after the weight DMAs (not prologue-critical)
        nc.sync.dma_start(out=xsA[:, 1, :], in_=x_ext[1, 0:128, :])
        nc.sync.dma_start(out=xsB[:, 1, :], in_=x_ext[1, 128:C, :])
        o_b0 = const.tile([128, 1], F32, name="ob0")
        o_b1 = const.tile([C - 128, 1], F32, name="ob1")
        nc.sync.dma_start(out=o_b0[:], in_=ob_ext[0:128, :])
        nc.sync.dma_start(out=o_b1[:], in_=ob_ext[128:C, :])
        ones64 = const.tile([128, HID], BF16, name="ones64")
        nc.sync.dma_start(out=ones64[:], in_=ones_ext[:])

        # ---- dw conv output tiles (bf16), ones row at partition 64 of B ----
        aq = {"A": xpool.tile([128, BPC, Lq], BF16, name="aqA"),
              "B": xpool.tile([C - 128, BPC, Lq], BF16, name="aqB")}
        xqb = {"A": xpool.tile([128, BPC, Lq], BF16, name="xqbA"),
               "B": xpool.tile([65, BPC, Lq], BF16, name="xqbB")}
        xkb = {"A": xpool.tile([128, BPC, Lkv], BF16, name="xkbA"),
               "B": xpool.tile([65, BPC, Lkv], BF16, name="xkbB")}
        xvb = {"A": xpool.tile([128, BPC, Lkv], BF16, name="xvbA"),
               "B": xpool.tile([65, BPC, Lkv], BF16, name="xvbB")}
        nc.sync.dma_start(out=xqb["B"][64:65, :, :], in_=onesq_ext[:])
        nc.sync.dma_start(out=xkb["B"][64:65, :, :], in_=oneskv_ext[:])
        nc.sync.dma_start(out=xvb["B"][64:65, :, :], in_=oneskv_ext[:])

        def emit_dw_q_taps(bi, tile_sel):
            # vector-engine taps for one tile half ("A" or "B")
            srcT, dst, dwt = ((xpA, aq["A"], dwA["q"]) if tile_sel == "A"
                              else (xpB, aq["B"], dwB["q"]))
            np_ = min(srcT.shape[0], 128)
            outap = dst[0:np_, bi].rearrange("p (h w) -> p h w", h=S)
            for t, (dy, dx) in enumerate(TAPS):
                inap = srcT[0:np_, bi, dy:dy + S, dx:dx + S]
                if t == 0:
                    nc.vector.tensor_scalar(outap, inap, dwt[:, 0:1],
                                            None, OP.mult)
                else:
                    nc.vector.scalar_tensor_tensor(
                        outap, inap, dwt[:, t:t + 1], outap,
                        OP.mult, OP.add)

        def emit_dw_q_reorder(bi, src_tiles, on_s=False):
            # reorder lq=(j*8+m) -> i=(m*128+j), contiguous dst
            for tl, srcT in src_tiles.items():
                np_ = 128 if tl == "A" else 64
                src_ap = srcT[0:np_, bi] if srcT.shape[1] == BPC else srcT
                src_ap = src_ap.rearrange("p (j m) -> p m j", m=8)
                dst_ap = xqb[tl][0:np_, bi].rearrange("p (m j) -> p m j", m=8)
                if on_s:
                    nc.scalar.copy(dst_ap, src_ap)
                else:
                    nc.vector.tensor_copy(dst_ap, src_ap)

        def emit_dw_q_pe(bi):
            # depthwise q conv as 9 accumulating diagonal matmuls (prologue:
            # PE is otherwise idle); returns psum tiles for the reorder copy
            qcpA = psw.tile([128, Lq], F32, name="work")
            qcpB = psw.tile([64, Lq], F32, name="work")
            for (qcp, qd, xp_t, np_) in ((qcpA, qdA, xpA, 128),
                                         (qcpB, qdB, xpB, 64)):
                for n in range(2):
                    rs = slice(16 * n, 16 * n + 16)
                    ns = slice(n * 512, (n + 1) * 512)
                    for t, (dy, dx) in enumerate(TAPS):
                        rhs = xp_t[0:np_, bi, dy + 16 * n:dy + 16 * n + 16,
                                   dx:dx + S]
                        nc.tensor.matmul(qcp[:, ns], qd[:, t, :], rhs,
                                         start=(t == 0), stop=(t == 8))
            return {"A": qcpA, "B": qcpB}

        def emit_dw_kv(bi, p):
            dst = xkb if p == "k" else xvb
            for ti, (phT, dwt) in enumerate(((phA, dwA[p]), (phB, dwB[p]))):
                np_ = min(phT.shape[0], 128)
                d = dst["A"] if ti == 0 else dst["B"]
                outap = d[0:np_, bi].rearrange("p (i j) -> p i j", i=Sk)
                for t, (dy, dx) in enumerate(TAPS):
                    inap = phT[0:np_, bi, dy % 2, dx % 2,
                               dy // 2:dy // 2 + 16, dx // 2:dx // 2 + 16]
                    if t == 0:
                        nc.vector.tensor_scalar(outap, inap, dwt[:, 0:1],
                                                None, OP.mult)
                    else:
                        nc.vector.scalar_tensor_tensor(
                            outap, inap, dwt[:, t:t + 1], outap,
                            OP.mult, OP.add)

        # ---- per-batch compute; q mms first so PE isn't gated on k/v taps --
        def pw_block(bi):
            q_sb, k_sb, vT_sb = [], [], []
            for hp in range(4):
                hs = slice(hp * 128, (hp + 1) * 128)
                qp = psw.tile([128, Lq], F32, name="work")
                for n in range(2):
                    ns = slice(n * 512, (n + 1) * 512)
                    nc.tensor.matmul(qp[:, ns], pwA["q"][:, hs],
                                     xqb["A"][:, bi, ns],
                                     start=True, stop=False)
                    nc.tensor.matmul(qp[:, ns], pwB["q"][:, hs],
                                     xqb["B"][:, bi, ns],
                                     start=False, stop=True)
                qs = wpool.tile([128, Lq], BF16, name=f"qsb{hp}", bufs=2)
                if bi == 0:
                    nc.scalar.copy(qs[:], qp[:])
                else:
                    nc.vector.tensor_copy(qs[:], qp[:])
                q_sb.append(qs)
            for hp in range(4):
                hs = slice(hp * 128, (hp + 1) * 128)
                kp = psw.tile([128, Lkv], F32, name="work")
                nc.tensor.matmul(kp[:], pwA["k"][:, hs],
                                 xkb["A"][:, bi, :],
                                 start=True, stop=False)
                nc.tensor.matmul(kp[:], pwB["k"][:, hs],
                                 xkb["B"][:, bi, :],
                                 start=False, stop=True)
                ks = wpool.tile([128, Lkv], BF16, name=f"ksb{hp}", bufs=2)
                nc.scalar.copy(ks[:], kp[:])
                k_sb.append(ks)
            for kc in range(2):
                vp = psw.tile([128, INNER], F32, name="work")
                kvs = slice(kc * 128, (kc + 1) * 128)
                nc.tensor.matmul(vp[:], xvb["A"][:, bi, kvs],
                                 pwA["v"][:], start=True, stop=False)
                nc.tensor.matmul(vp[:], xvb["B"][:, bi, kvs],
                                 pwB["v"][:], start=False, stop=True)
                vs = wpool.tile([128, INNER], BF16, name=f"vtsb{kc}", bufs=2)
                nc.scalar.copy(vs[:], vp[:])
                vT_sb.append(vs)
            return q_sb, k_sb, vT_sb

        Fs = {}

        def attn_pair(bi, hp, q_sb, k_sb, vT_sb):
            if hp == 0:
                Fs[bi] = wpool.tile([128, 4, Lq], BF16, name="Ftile", bufs=2)
            F = Fs[bi]
            h0, h1 = 2 * hp, 2 * hp + 1
            qs_t, ks_t = q_sb[hp], k_sb[hp]
            expAs = {}
            for idx, h in enumerate((h0, h1)):
                R = slice(idx * HID, (idx + 1) * HID)
                for kc in range(2):
                    st = psw.tile([128, Lq], F32, name="work")
                    kvs = slice(kc * 128, (kc + 1) * 128)
                    for n in range(2):
                        ns = slice(n * 512, (n + 1) * 512)
                        nc.tensor.matmul(st[:, ns],
                                         ks_t[R, kvs],
                                         qs_t[R, ns],
                                         start=True, stop=True)
                    ex = wpool.tile([128, Lq], BF16, name="expA", bufs=6)
                    nc.scalar.activation(ex[:], st[:], AF.Exp,
                                         scale=1.0 / (HID ** 0.5))
                    expAs[(h, kc)] = ex

            avp = psw.tile([128, Lq], F32, name="work")
            dnp = psw.tile([128, Lq], F32, name="work")
            for idx, h in enumerate((h0, h1)):
                tp = (0, 0) if idx == 0 else (0, 64)
                rows = slice(idx * HID, (idx + 1) * HID)
                hs = slice(h * HID, (h + 1) * HID)
                for kc in range(2):
                    for n in range(2):
                        ns = slice(n * 512, (n + 1) * 512)
                        nc.tensor.matmul(avp[rows, ns],
                                         vT_sb[kc][:, hs],
                                         expAs[(h, kc)][:, ns],
                                         start=(kc == 0), stop=(kc == 1),
                                         tile_position=tp)
                        nc.tensor.matmul(dnp[rows, ns], ones64[:, :],
                                         expAs[(h, kc)][:, ns],
                                         start=(kc == 0), stop=(kc == 1),
                                         tile_position=tp)
            rc = wpool.tile([128, Lq], F32, name="recip", bufs=2)
            nc.vector.reciprocal_approx_fast(rc[:], dnp[:])
            avbf = wpool.tile([128, Lq], BF16, name="avbf", bufs=2)
            nc.vector.tensor_tensor(avbf[:], avp[:], rc[:], OP.mult)

            # refold avbf -> F[kk][par*64+d, h*128+r]; GpSimd for the slack
            # pair, Vector for the half-closing pair (drains into o_proj)
            on_v = hp % 2 == 1
            for idx, h in enumerate((h0, h1)):
                rows = slice(idx * HID, (idx + 1) * HID)
                src4 = avbf[rows, :].rearrange("p (a b j) -> p a b j",
                                               a=4, b=2)
                for par in range(2):
                    dst = F[par * HID:(par + 1) * HID, :,
                            h * 128:(h + 1) * 128]
                    if on_v:
                        nc.vector.tensor_copy(dst, src4[:, :, par, :])
                    else:
                        nc.gpsimd.tensor_copy(dst, src4[:, :, par, :])

        def o_proj_half(bi, half):
            cs = slice(half * 512, (half + 1) * 512)
            P0 = psw.tile([128, 512], F32, name="work")
            P1 = psw.tile([C - 128, 512], F32, name="work")
            for kk in range(4):
                nc.tensor.matmul(P0[:], owt4[:, kk, 0:128],
                                 Fs[bi][:, kk, cs],
                                 start=(kk == 0), stop=(kk == 3))
                nc.tensor.matmul(P1[:], owt4[:, kk, 128:C],
                                 Fs[bi][:, kk, cs],
                                 start=(kk == 0), stop=(kk == 3))
            os0 = wpool.tile([128, 512], F32, name="os0", bufs=2)
            os1 = wpool.tile([C - 128, 512], F32, name="os1", bufs=2)
            nc.scalar.activation(os0[:], P0[:], AF.Identity, bias=o_b0[:])
            nc.scalar.activation(os1[:], P1[:], AF.Identity, bias=o_b1[:])
            nc.sync.dma_start(out=out_ext[bi, 0:128, cs], in_=os0[:])
            nc.sync.dma_start(out=out_ext[bi, 128:C, cs], in_=os1[:])

        # pipeline: batch-0 q conv on the (idle) PE during prologue; batch-1
        # dw emitted in small chunks between batch-0 attention pairs so
        # pair-critical V ops aren't stuck behind long tap chains.
        qcp0 = emit_dw_q_pe(0)
        emit_dw_kv(0, "k")
        emit_dw_q_reorder(0, qcp0, on_s=True)
        emit_dw_kv(0, "v")
        t0 = pw_block(0)
        emit_convert(1)
        emit_planes(1)
        attn_pair(0, 0, *t0)
        emit_dw_q_taps(1, "A")
        attn_pair(0, 1, *t0)
        o_proj_half(0, 0)
        emit_dw_q_taps(1, "B")
        attn_pair(0, 2, *t0)
        emit_dw_q_reorder(1, aq)
        emit_dw_kv(1, "k")
        attn_pair(0, 3, *t0)
        o_proj_half(0, 1)
        emit_dw_kv(1, "v")
        t1 = pw_block(1)
        attn_pair(1, 0, *t1)
        attn_pair(1, 1, *t1)
        o_proj_half(1, 0)
        attn_pair(1, 2, *t1)
        attn_pair(1, 3, *t1)
        o_proj_half(1, 1)

    nc.finalize()
    return nc


def _prep_weights(inputs):
    g = lambda k: np.asarray(inputs[k], np.float32)
    w = {}
    for p in ("q", "k", "v"):
        scale = g(p + "_bn_g") / np.sqrt(g(p + "_bn_v") + EPS)
        dww = g(p + "_dw")[:, 0].reshape(C, 9) * scale[:, None]
        biasc = g(p + "_bn_b") - g(p + "_bn_m") * scale
        pwm = g(p + "_pw")[:, :, 0, 0]
        const_row = pwm @ biasc
        w[p + "pwT"] = np.ascontiguousarray(
            np.concatenate([pwm.T, const_row[None, :]], 0)).astype(BF16NP)
        w[p + "dw"] = np.ascontiguousarray(dww)
    dq = w["qdw"]                                     # [C, 9] fused dw weights
    qdA = np.zeros((128, 9, 128), np.float32)
    qdA[np.arange(128), :, np.arange(128)] = dq[0:128]
    qdB = np.zeros((64, 9, 64), np.float32)
    qdB[np.arange(64), :, np.arange(64)] = dq[128:C]
    w["qdiagA"] = np.ascontiguousarray(qdA.reshape(128, 9 * 128)).astype(BF16NP)
    w["qdiagB"] = np.ascontiguousarray(qdB.reshape(64, 9 * 64)).astype(BF16NP)
    owt = g("o_w")[:, :, 0, 0].T                      # [INNER, C]
    w["owt4"] = np.ascontiguousarray(
        owt.reshape(4, 128, C).transpose(1, 0, 2).reshape(128, 4 * C)
    ).astype(BF16NP)
    w["ones128x64"] = np.ones((128, HID), BF16NP)
    w["onesq"] = np.ones((1, BPC, Lq), BF16NP)
    w["oneskv"] = np.ones((1, BPC, Lkv), BF16NP)
    w["ob"] = np.ascontiguousarray(g("o_b")[:, None])
    return w


def kernel(**inputs):
    global _NC, LAST_RESULT
    if _NC is None:
        _NC = _build()
    w = _prep_weights(inputs)
    x = np.ascontiguousarray(
        np.asarray(inputs["x"], np.float32).reshape(B, C, Lq))
    in_maps = []
    for c in range(NCORES):
        m = {"x": np.ascontiguousarray(x[c * BPC:(c + 1) * BPC])}
        m.update(w)
        in_maps.append(m)
    res = run_bass_kernel_spmd(_NC, in_maps, list(range(NCORES)))
    LAST_RESULT = res
    out = np.concatenate([r["out"] for r in res.results], 0)
    return np.ascontiguousarray(out.reshape(B, C, S, S).astype(np.float32))


# revision 34
# speedup vs baseline: 1.3315x; 1.3315x over previous
"""Trainium2 Bass kernel for nn_Attention_16071767621814.

MobileViT-style attention block: 3x (depthwise3x3 conv + BN + 1x1 pointwise)
for q/k/v, 8-head attention (Lq=1024, Lkv=256, d=64), head-mixing reshape,
1x1 output projection.

Sharding: pure data-parallel over batch (16 batches / 8 cores = 2 per core),
zero collectives.

Per-core design (all layouts [feature-on-partition, token-on-free]):
- x DMA'd contiguously to a staging tile, then pad-converted to bf16 on
  Scalar/GpSimd (avoids fragmented padded-interior DMA descriptors).
- BN folded host-side: scale into dw weights, bias via pw const row.
- depthwise convs as 9 shifted scalar_tensor_tensor taps in bf16 on Vector.
- q tokens ordered i = m*128 + j (lq = j*8 + m) so the head-mixing
  reshape's m-phases are contiguous 128-blocks.
- pw matmuls at M=128 (two heads per matmul).
- S^T = k^T q via PE, exp on ScalarE, denominator via ones-lhsT matmul
  broadcast, normalize (avp*rc) on GpSimd -> avbf bf16.
- avbf refolded into F[kk][par*64+d, h*128+r] via 4 SBUF->SBUF DMAs/pair;
  o_proj is then 16 matmuls of N=512,K=128 per batch from F.
"""

import numpy as np
import ml_dtypes
BF16NP = ml_dtypes.bfloat16

from concourse import bass, bacc, tile, mybir
from concourse.bass_utils import run_bass_kernel_spmd

F32 = mybir.dt.float32
BF16 = mybir.dt.bfloat16
AF = mybir.ActivationFunctionType
OP = mybir.AluOpType

NCORES = 8
B, C, S = 16, 192, 32
BPC = B // NCORES          # 2 batches per core
Lq = S * S                 # 1024
Sk = S // 2                # 16
Lkv = Sk * Sk              # 256
HEADS, HID, INNER = 8, 64, 512
EPS = 1e-5
PS = S + 2                 # padded spatial 34

_NC = None
LAST_RESULT = None


def _build():
    nc = bacc.Bacc("TRN2", target_bir_lowering=False, debug=False,
                   num_devices=NCORES)

    x_ext = nc.declare_dram_parameter("x", [BPC, C, Lq], F32, isOutput=False)
    pw_ext = {}
    dw_ext = {}
    for p in ("q", "k", "v"):
        pw_ext[p] = nc.declare_dram_parameter(p + "pwT", [C + 1, INNER], BF16,
                                              isOutput=False)
        dw_ext[p] = nc.declare_dram_parameter(p + "dw", [C, 9], F32,
                                              isOutput=False)
    owt4_ext = nc.declare_dram_parameter("owt4", [128, 4 * C], BF16,
                                         isOutput=False)
    qdA_ext = nc.declare_dram_parameter("qdiagA", [128, 9 * 128], BF16,
                                        isOutput=False)
    qdB_ext = nc.declare_dram_parameter("qdiagB", [64, 9 * 64], BF16,
                                        isOutput=False)
    ones_ext = nc.declare_dram_parameter("ones128x64", [128, HID], BF16,
                                         isOutput=False)
    onesq_ext = nc.declare_dram_parameter("onesq", [1, BPC, Lq], BF16,
                                          isOutput=False)
    oneskv_ext = nc.declare_dram_parameter("oneskv", [1, BPC, Lkv], BF16,
                                           isOutput=False)
    ob_ext = nc.declare_dram_parameter("ob", [C, 1], F32, isOutput=False)
    out_ext = nc.declare_dram_parameter("out", [BPC, C, Lq], F32, isOutput=True)

    TAPS = [(dy, dx) for dy in range(3) for dx in range(3)]

    from contextlib import ExitStack
    with tile.TileContext(nc) as tc, ExitStack() as ctx:
        const = ctx.enter_context(tc.tile_pool(name="const", bufs=1))
        xpool = ctx.enter_context(tc.tile_pool(name="xpool", bufs=1))
        wpool = ctx.enter_context(tc.tile_pool(name="wpool", bufs=2))
        psw = ctx.enter_context(tc.tile_pool(name="psw", bufs=4, space="PSUM"))

        # ---- x staging DMA (contiguous, fast); batch 0 + diag weights
        # first so the prologue diag-conv isn't starved ----
        xsA = xpool.tile([128, BPC, Lq], F32, name="xsA")
        xsB = xpool.tile([C - 128, BPC, Lq], F32, name="xsB")
        nc.sync.dma_start(out=xsA[:, 0, :], in_=x_ext[0, 0:128, :])
        nc.sync.dma_start(out=xsB[:, 0, :], in_=x_ext[0, 128:C, :])
        qdA = const.tile([128, 9, 128], BF16, name="qdA")
        nc.sync.dma_start(out=qdA[:],
                          in_=qdA_ext[:].rearrange("p (t c) -> p t c", t=9))
        qdB = const.tile([64, 9, 64], BF16, name="qdB")
        nc.sync.dma_start(out=qdB[:],
                          in_=qdB_ext[:].rearrange("p (t c) -> p t c", t=9))

        # padded bf16 input tiles (zero borders only; interior overwritten)
        xpA = xpool.tile([128, BPC, PS, PS], BF16, name="xpA")
        xpB = xpool.tile([C - 128, BPC, PS, PS], BF16, name="xpB")
        for xp_t in (xpA, xpB):
            nc.vector.memset(xp_t[:, :, 0:1, :], 0.0)
            nc.vector.memset(xp_t[:, :, S + 1:S + 2, :], 0.0)
            nc.vector.memset(xp_t[:, :, :, 0:1], 0.0)
            nc.vector.memset(xp_t[:, :, :, S + 1:S + 2], 0.0)

        # phase-split planes for the stride-2 k/v convs: P[c,bi,py,px,u,v] =
        # xpad[c, bi, 2u+py, 2v+px]  (17 used rows, 18-wide for even stride)
        PU, PV = 17, 18
        phA = xpool.tile([128, BPC, 2, 2, PU, PV], BF16, name="phA")
        phB = xpool.tile([C - 128, BPC, 2, 2, PU, PV], BF16, name="phB")
        for ph_t in (phA, phB):
            # only pad row u=0 (py=0 planes) and pad col v=0 (px=0 planes)
            # are ever read by taps; zero just those.
            nc.vector.memset(ph_t[:, :, 0, :, 0:1, :], 0.0)
            nc.vector.memset(ph_t[:, :, :, 0, :, 0:1], 0.0)

        def emit_convert(bi):
            nc.scalar.copy(
                xpA[:, bi, 1:S + 1, 1:S + 1],
                xsA[:, bi].rearrange("p (h w) -> p h w", h=S))
            nc.scalar.copy(
                xpB[:, bi, 1:S + 1, 1:S + 1],
                xsB[:, bi].rearrange("p (h w) -> p h w", h=S))

        def emit_planes(bi):
            for (xs_t, ph_t) in ((xsA, phA), (xsB, phB)):
                np_ = min(xs_t.shape[0], 128)
                src = xs_t[0:np_, bi].rearrange("p (h w) -> p h w", h=S)
                for py in range(2):
                    for px in range(2):
                        u0 = 1 - py
                        v0 = 1 - px
                        r0 = 1 if py == 0 else 0
                        c0 = 1 if px == 0 else 0
                        dst = ph_t[0:np_, bi, py, px, u0:u0 + 16, v0:v0 + 16]
                        srcv = src[:, r0:r0 + 31:2, c0:c0 + 31:2]
                        nc.scalar.copy(dst, srcv)

        emit_convert(0)
        emit_planes(0)

        # ---- weights to SBUF ----
        pwA, pwB, dwA, dwB = {}, {}, {}, {}
        for p in ("q", "k", "v"):
            pwA[p] = const.tile([128, INNER], BF16, name=f"pwA{p}")
            pwB[p] = const.tile([C + 1 - 128, INNER], BF16, name=f"pwB{p}")
            nc.sync.dma_start(out=pwA[p][:], in_=pw_ext[p][0:128, :])
            nc.sync.dma_start(out=pwB[p][:], in_=pw_ext[p][128:C + 1, :])
            dwA[p] = const.tile([128, 9], F32, name=f"dwA{p}")
            dwB[p] = const.tile([C - 128, 9], F32, name=f"dwB{p}")
            nc.sync.dma_start(out=dwA[p][:], in_=dw_ext[p][0:128, :])
            nc.sync.dma_start(out=dwB[p][:], in_=dw_ext[p][128:C, :])
        owt4 = const.tile([128, 4, C], BF16, name="owt4")
        nc.sync.dma_start(out=owt4[:],
                          in_=owt4_ext[:].rearrange("p (k c) -> p k c", k=4))
        # batch-1 staging after the weight DMAs (not prologue-critical)
        nc.sync.dma_start(out=xsA[:, 1, :], in_=x_ext[1, 0:128, :])
        nc.sync.dma_start(out=xsB[:, 1, :], in_=x_ext[1, 128:C, :])
        o_b0 = const.tile([128, 1], F32, name="ob0")
        o_b1 = const.tile([C - 128, 1], F32, name="ob1")
        nc.sync.dma_start(out=o_b0[:], in_=ob_ext[0:128, :])
        nc.sync.dma_start(out=o_b1[:], in_=ob_ext[128:C, :])
        ones64 = const.tile([128, HID], BF16, name="ones64")
        nc.sync.dma_start(out=ones64[:], in_=ones_ext[:])

        # ---- dw conv output tiles (bf16), ones row at partition 64 of B ----
        aq = {"A": xpool.tile([128, BPC, Lq], BF16, name="aqA"),
              "B": xpool.tile([C - 128, BPC, Lq], BF16, name="aqB")}
        xqb = {"A": xpool.tile([128, BPC, Lq], BF16, name="xqbA"),
               "B": xpool.tile([65, BPC, Lq], BF16, name="xqbB")}
        xkb = {"A": xpool.tile([128, BPC, Lkv], BF16, name="xkbA"),
               "B": xpool.tile([65, BPC, Lkv], BF16, name="xkbB")}
        xvb = {"A": xpool.tile([128, BPC, Lkv], BF16, name="xvbA"),
               "B": xpool.tile([65, BPC, Lkv], BF16, name="xvbB")}
        nc.sync.dma_start(out=xqb["B"][64:65, :, :], in_=onesq_ext[:])
        nc.sync.dma_start(out=xkb["B"][64:65, :, :], in_=oneskv_ext[:])
        nc.sync.dma_start(out=xvb["B"][64:65, :, :], in_=oneskv_ext[:])

        def emit_dw_q_taps(bi, tile_sel):
            # vector-engine taps for one tile half ("A" or "B")
            srcT, dst, dwt = ((xpA, aq["A"], dwA["q"]) if tile_sel == "A"
                              else (xpB, aq["B"], dwB["q"]))
            np_ = min(srcT.shape[0], 128)
            outap = dst[0:np_, bi].rearrange("p (h w) -> p h w", h=S)
            for t, (dy, dx) in enumerate(TAPS):
                inap = srcT[0:np_, bi, dy:dy + S, dx:dx + S]
                if t == 0:
                    nc.vector.tensor_scalar(outap, inap, dwt[:, 0:1],
                                            None, OP.mult)
                else:
                    nc.vector.scalar_tensor_tensor(
                        outap, inap, dwt[:, t:t + 1], outap,
                        OP.mult, OP.add)

        def emit_dw_q_reorder(bi, src_tiles, on_s=False):
            # reorder lq=(j*8+m) -> i=(m*128+j), contiguous dst
            for tl, srcT in src_tiles.items():
                np_ = 128 if tl == "A" else 64
                src_ap = srcT[0:np_, bi] if srcT.shape[1] == BPC else srcT
                src_ap = src_ap.rearrange("p (j m) -> p m j", m=8)
                dst_ap = xqb[tl][0:np_, bi].rearrange("p (m j) -> p m j", m=8)
                if on_s:
                    nc.scalar.copy(dst_ap, src_ap)
                else:
                    nc.vector.tensor_copy(dst_ap, src_ap)

        def emit_dw_q_pe(bi):
            # depthwise q conv as 9 accumulating diagonal matmuls (prologue:
            # PE is otherwise idle); returns psum tiles for the reorder copy
            qcpA = psw.tile([128, Lq], F32, name="work")
            qcpB = psw.tile([64, Lq], F32, name="work")
            for (qcp, qd, xp_t, np_) in ((qcpA, qdA, xpA, 128),
                                         (qcpB, qdB, xpB, 64)):
                for n in range(2):
                    rs = slice(16 * n, 16 * n + 16)
                    ns = slice(n * 512, (n + 1) * 512)
                    for t, (dy, dx) in enumerate(TAPS):
                        rhs = xp_t[0:np_, bi, dy + 16 * n:dy + 16 * n + 16,
                                   dx:dx + S]
                        nc.tensor.matmul(qcp[:, ns], qd[:, t, :], rhs,
                                         start=(t == 0), stop=(t == 8))
            return {"A": qcpA, "B": qcpB}

        def emit_dw_kv(bi, p):
            dst = xkb if p == "k" else xvb
            for ti, (phT, dwt) in enumerate(((phA, dwA[p]), (phB, dwB[p]))):
                np_ = min(phT.shape[0], 128)
                d = dst["A"] if ti == 0 else dst["B"]
                outap = d[0:np_, bi].rearrange("p (i j) -> p i j", i=Sk)
                for t, (dy, dx) in enumerate(TAPS):
                    inap = phT[0:np_, bi, dy % 2, dx % 2,
                               dy // 2:dy // 2 + 16, dx // 2:dx // 2 + 16]
                    if t == 0:
                        nc.vector.tensor_scalar(outap, inap, dwt[:, 0:1],
                                                None, OP.mult)
                    else:
                        nc.vector.scalar_tensor_tensor(
                            outap, inap, dwt[:, t:t + 1], outap,
                            OP.mult, OP.add)

        # ---- per-batch compute; q mms first so PE isn't gated on k/v taps --
        def pw_block(bi):
            q_sb, k_sb, vT_sb = [], [], []
            for hp in range(4):
                hs = slice(hp * 128, (hp + 1) * 128)
                qp = psw.tile([128, Lq], F32, name="work")
                for n in range(2):
                    ns = slice(n * 512, (n + 1) * 512)
                    nc.tensor.matmul(qp[:, ns], pwA["q"][:, hs],
                                     xqb["A"][:, bi, ns],
                                     start=True, stop=False)
                    nc.tensor.matmul(qp[:, ns], pwB["q"][:, hs],
                                     xqb["B"][:, bi, ns],
                                     start=False, stop=True)
                qs = wpool.tile([128, Lq], BF16, name=f"qsb{hp}", bufs=2)
                nc.scalar.copy(qs[:], qp[:])
                q_sb.append(qs)
            for hp in range(4):
                hs = slice(hp * 128, (hp + 1) * 128)
                kp = psw.tile([128, Lkv], F32, name="work")
                nc.tensor.matmul(kp[:], pwA["k"][:, hs],
                                 xkb["A"][:, bi, :],
                                 start=True, stop=False)
                nc.tensor.matmul(kp[:], pwB["k"][:, hs],
                                 xkb["B"][:, bi, :],
                                 start=False, stop=True)
                ks = wpool.tile([128, Lkv], BF16, name=f"ksb{hp}", bufs=2)
                nc.scalar.copy(ks[:], kp[:])
                k_sb.append(ks)
            for kc in range(2):
                vp = psw.tile([128, INNER], F32, name="work")
                kvs = slice(kc * 128, (kc + 1) * 128)
                nc.tensor.matmul(vp[:], xvb["A"][:, bi, kvs],
                                 pwA["v"][:], start=True, stop=False)
                nc.tensor.matmul(vp[:], xvb["B"][:, bi, kvs],
                                 pwB["v"][:], start=False, stop=True)
                vs = wpool.tile([128, INNER], BF16, name=f"vtsb{kc}", bufs=2)
                nc.scalar.copy(vs[:], vp[:])
                vT_sb.append(vs)
            return q_sb, k_sb, vT_sb

        Fs = {}

        def attn_st(bi, hp, q_sb, k_sb, vT_sb):
            if hp == 0:
                Fs[bi] = wpool.tile([128, 4, Lq], BF16, name="Ftile", bufs=2)
            h0, h1 = 2 * hp, 2 * hp + 1
            qs_t, ks_t = q_sb[hp], k_sb[hp]
            expAs = {}
            for idx, h in enumerate((h0, h1)):
                R = slice(idx * HID, (idx + 1) * HID)
                for kc in range(2):
                    st = psw.tile([128, Lq], F32, name="work")
                    kvs = slice(kc * 128, (kc + 1) * 128)
                    for n in range(2):
                        ns = slice(n * 512, (n + 1) * 512)
                        nc.tensor.matmul(st[:, ns],
                                         ks_t[R, kvs],
                                         qs_t[R, ns],
                                         start=True, stop=True)
                    ex = wpool.tile([128, Lq], BF16, name="expA", bufs=8)
                    nc.scalar.activation(ex[:], st[:], AF.Exp,
                                         scale=1.0 / (HID ** 0.5))
                    expAs[(h, kc)] = ex
            return expAs

        def attn_av(bi, hp, expAs, q_sb, k_sb, vT_sb):
            F = Fs[bi]
            h0, h1 = 2 * hp, 2 * hp + 1
            avp = psw.tile([128, Lq], F32, name="work")
            dnp = psw.tile([128, Lq], F32, name="work")
            for idx, h in enumerate((h0, h1)):
                tp = (0, 0) if idx == 0 else (0, 64)
                rows = slice(idx * HID, (idx + 1) * HID)
                hs = slice(h * HID, (h + 1) * HID)
                for kc in range(2):
                    for n in range(2):
                        ns = slice(n * 512, (n + 1) * 512)
                        nc.tensor.matmul(avp[rows, ns],
                                         vT_sb[kc][:, hs],
                                         expAs[(h, kc)][:, ns],
                                         start=(kc == 0), stop=(kc == 1),
                                         tile_position=tp)
                        nc.tensor.matmul(dnp[rows, ns], ones64[:, :],
                                         expAs[(h, kc)][:, ns],
                                         start=(kc == 0), stop=(kc == 1),
                                         tile_position=tp)
            rc = wpool.tile([128, Lq], F32, name="recip", bufs=2)
            nc.vector.reciprocal_approx_fast(rc[:], dnp[:])
            avbf = wpool.tile([128, Lq], BF16, name="avbf", bufs=2)
            nc.vector.tensor_tensor(avbf[:], avp[:], rc[:], OP.mult)

            # refold avbf -> F[kk][par*64+d, h*128+r]; GpSimd for the slack
            # pair, Vector for the half-closing pair (drains into o_proj)
            on_v = hp % 2 == 1
            for idx, h in enumerate((h0, h1)):
                rows = slice(idx * HID, (idx + 1) * HID)
                src4 = avbf[rows, :].rearrange("p (a b j) -> p a b j",
                                               a=4, b=2)
                for par in range(2):
                    dst = F[par * HID:(par + 1) * HID, :,
                            h * 128:(h + 1) * 128]
                    if on_v:
                        nc.vector.tensor_copy(dst, src4[:, :, par, :])
                    else:
                        nc.scalar.copy(dst, src4[:, :, par, :])

        def o_proj_half(bi, half):
            cs = slice(half * 512, (half + 1) * 512)
            P0 = psw.tile([128, 512], F32, name="work")
            P1 = psw.tile([C - 128, 512], F32, name="work")
            for kk in range(4):
                nc.tensor.matmul(P0[:], owt4[:, kk, 0:128],
                                 Fs[bi][:, kk, cs],
                                 start=(kk == 0), stop=(kk == 3))
                nc.tensor.matmul(P1[:], owt4[:, kk, 128:C],
                                 Fs[bi][:, kk, cs],
                                 start=(kk == 0), stop=(kk == 3))
            os0 = wpool.tile([128, 512], F32, name="os0", bufs=2)
            os1 = wpool.tile([C - 128, 512], F32, name="os1", bufs=2)
            nc.scalar.activation(os0[:], P0[:], AF.Identity, bias=o_b0[:])
            nc.scalar.activation(os1[:], P1[:], AF.Identity, bias=o_b1[:])
            nc.sync.dma_start(out=out_ext[bi, 0:128, cs], in_=os0[:])
            nc.sync.dma_start(out=out_ext[bi, 128:C, cs], in_=os1[:])

        # pipeline: batch-0 q conv on the (idle) PE during prologue; batch-1
        # dw emitted in small chunks between batch-0 attention pairs so
        # pair-critical V ops aren't stuck behind long tap chains.
        qcp0 = emit_dw_q_pe(0)
        emit_dw_kv(0, "k")
        emit_dw_q_reorder(0, qcp0, on_s=True)
        emit_dw_kv(0, "v")
        t0 = pw_block(0)
        emit_convert(1)
        emit_planes(1)
        e00 = attn_st(0, 0, *t0)
        e01 = attn_st(0, 1, *t0)
        attn_av(0, 0, e00, *t0)
        emit_dw_q_taps(1, "A")
        e02 = attn_st(0, 2, *t0)
        attn_av(0, 1, e01, *t0)
        o_proj_half(0, 0)
        emit_dw_q_taps(1, "B")
        e03 = attn_st(0, 3, *t0)
        attn_av(0, 2, e02, *t0)
        emit_dw_q_reorder(1, aq)
        emit_dw_kv(1, "k")
        attn_av(0, 3, e03, *t0)
        o_proj_half(0, 1)
        emit_dw_kv(1, "v")
        t1 = pw_block(1)
        e10 = attn_st(1, 0, *t1)
        e11 = attn_st(1, 1, *t1)
        attn_av(1, 0, e10, *t1)
        e12 = attn_st(1, 2, *t1)
        attn_av(1, 1, e11, *t1)
        o_proj_half(1, 0)
        e13 = attn_st(1, 3, *t1)
        attn_av(1, 2, e12, *t1)
        attn_av(1, 3, e13, *t1)
        o_proj_half(1, 1)

    nc.finalize()
    return nc


def _prep_weights(inputs):
    g = lambda k: np.asarray(inputs[k], np.float32)
    w = {}
    for p in ("q", "k", "v"):
        scale = g(p + "_bn_g") / np.sqrt(g(p + "_bn_v") + EPS)
        dww = g(p + "_dw")[:, 0].reshape(C, 9) * scale[:, None]
        biasc = g(p + "_bn_b") - g(p + "_bn_m") * scale
        pwm = g(p + "_pw")[:, :, 0, 0]
        const_row = pwm @ biasc
        w[p + "pwT"] = np.ascontiguousarray(
            np.concatenate([pwm.T, const_row[None, :]], 0)).astype(BF16NP)
        w[p + "dw"] = np.ascontiguousarray(dww)
    dq = w["qdw"]                                     # [C, 9] fused dw weights
    qdA = np.zeros((128, 9, 128), np.float32)
    qdA[np.arange(128), :, np.arange(128)] = dq[0:128]
    qdB = np.zeros((64, 9, 64), np.float32)
    qdB[np.arange(64), :, np.arange(64)] = dq[128:C]
    w["qdiagA"] = np.ascontiguousarray(qdA.reshape(128, 9 * 128)).astype(BF16NP)
    w["qdiagB"] = np.ascontiguousarray(qdB.reshape(64, 9 * 64)).astype(BF16NP)
    owt = g("o_w")[:, :, 0, 0].T                      # [INNER, C]
    w["owt4"] = np.ascontiguousarray(
        owt.reshape(4, 128, C).transpose(1, 0, 2).reshape(128, 4 * C)
    ).astype(BF16NP)
    w["ones128x64"] = np.ones((128, HID), BF16NP)
    w["onesq"] = np.ones((1, BPC, Lq), BF16NP)
    w["oneskv"] = np.ones((1, BPC, Lkv), BF16NP)
    w["ob"] = np.ascontiguousarray(g("o_b")[:, None])
    return w


def kernel(**inputs):
    global _NC, LAST_RESULT
    if _NC is None:
        _NC = _build()
    w = _prep_weights(inputs)
    x = np.ascontiguousarray(
        np.asarray(inputs["x"], np.float32).reshape(B, C, Lq))
    in_maps = []
    for c in range(NCORES):
        m = {"x": np.ascontiguousarray(x[c * BPC:(c + 1) * BPC])}
        m.update(w)
        in_maps.append(m)
    res = run_bass_kernel_spmd(_NC, in_maps, list(range(NCORES)))
    LAST_RESULT = res
    out = np.concatenate([r["out"] for r in res.results], 0)
    return np.ascontiguousarray(out.reshape(B, C, S, S).astype(np.float32))
